# revision 10
# baseline (speedup 1.0000x reference)
"""NerfHead Trainium2 kernel: per-sample generated 2-layer MLP over pixels.

Sharding: pure data parallel over the batch dim across 8 cores.
Device works in d-major ("transposed") pixel layout; host does all layout
permutations (legal: host only re-stages inputs/outputs, all FLOPs on device).
"""
import sys
from collections import deque
from contextlib import ExitStack

import ml_dtypes
import numpy as np

sys.path.insert(0, "/opt/trn_rl_repo")

import concourse.bass as bass  # noqa: E402
import concourse.tile as tile  # noqa: E402
from concourse import bacc, mybir  # noqa: E402

BF16 = mybir.dt.bfloat16
F32 = mybir.dt.float32
F32R = mybir.dt.float32r
AF = mybir.ActivationFunctionType

RMS_EPS = 1.1920928955078125e-07

N_CORES = 8
BS = 2048
NPIX = 256
D = 128
PD = 768  # patch_dim
KC = PD // 128  # 6 contraction chunks
NJ = 2 * D  # 256 j-tiles total (128 per half)
JG = 8  # j-tiles per W slab


def build_program(B, use_silu=True):
    """Build the per-core Bass program for a shard of B samples."""
    nc = bacc.Bacc("TRN2", target_bir_lowering=False, debug=False,
                   num_devices=N_CORES)

    pixT_d = nc.dram_tensor("pixT", (B, D, NPIX), BF16, kind="ExternalInput")
    patT_d = nc.dram_tensor("patchesT", (PD, B), BF16, kind="ExternalInput")
    w_d = nc.dram_tensor("W", (2, PD, D * D), BF16, kind="ExternalInput")
    bias_d = nc.dram_tensor("Bias", (2, D, D), F32, kind="ExternalInput")
    nw_d = nc.dram_tensor("normw", (1, D), BF16, kind="ExternalInput")
    outT_d = nc.dram_tensor("outT", (B, D, NPIX), F32, kind="ExternalOutput")

    with tile.TileContext(nc) as tc, ExitStack() as ctx:
        const = ctx.enter_context(tc.tile_pool(name="const", bufs=1))
        bigp = ctx.enter_context(tc.tile_pool(name="big", bufs=1))

        # constants / persistent tiles
        pats = []
        for k in range(KC):
            t = const.tile([128, B], BF16, tag=f"pat{k}")
            nc.sync.dma_start(t[:], patT_d.ap()[k * 128:(k + 1) * 128, :])
            pats.append(t)
        bt = const.tile([D, 2, D], F32, tag="bias")
        nc.sync.dma_start(bt[:], bias_d.ap().rearrange("h i j -> i h j"))
        nw = const.tile([1, D], BF16, tag="normw")
        nc.sync.dma_start(nw[:], nw_d.ap())
        maskones = const.tile([128, 2 * D + 1], BF16, tag="maskones")
        nc.vector.memset(maskones[:], 0.0)
        nc.vector.memset(maskones[:, D:D + 1], 1.0)
        onescol = const.tile([128, 1], BF16, tag="onescol")
        nc.vector.memset(onescol[:], 1.0)
        epsb = const.tile([1, 1], F32, tag="epsb")
        nc.vector.memset(epsb[:], RMS_EPS)
        inv_cn = const.tile([D, B], F32, tag="invcn")
        cn_tmp = const.tile([D, B], F32, tag="cntmp")

        big = bigp.tile([128, NJ, B], BF16, tag="big")

        # ---- Phase 1: params generation ----
        with tc.tile_pool(name="wslab", bufs=2 * KC) as wpool, \
             tc.tile_pool(name="sq1", bufs=3) as sqpool, \
             tc.tile_pool(name="mm1ps", bufs=4, space="PSUM") as mm1ps, \
             tc.tile_pool(name="cn2ps", bufs=1, space="PSUM") as cn2ps:
            cn2 = cn2ps.tile([D, B], F32, tag="cn2")
            pending = deque()  # (j, sq_tile) for delayed cn2 matmuls

            def emit_cn2(j, sq):
                nc.tensor.matmul(cn2[:], maskones[:, D - j:2 * D - j], sq[:],
                                 start=(j == 0), stop=(j == D - 1))

            for half in range(2):
                for jg in range(D // JG):
                    slabs = []
                    for k in range(KC):
                        s = wpool.tile([128, JG * 128], BF16, tag="wslab")
                        nc.sync.dma_start(
                            s[:], w_d.ap()[half, k * 128:(k + 1) * 128,
                                           jg * JG * 128:(jg + 1) * JG * 128])
                        slabs.append(s)
                    for jl in range(JG):
                        j = jg * JG + jl
                        jj = half * D + j
                        ps = mm1ps.tile([D, B], F32, tag="mm1")
                        for k in range(KC):
                            nc.tensor.matmul(
                                ps[:], slabs[k][:, jl * 128:(jl + 1) * 128],
                                pats[k][:], start=(k == 0), stop=(k == KC - 1))
                        nc.scalar.activation(big[:, jj, :], ps[:], AF.Identity,
                                             bias=bt[:, half, j:j + 1])
                        if half == 0:
                            sq = sqpool.tile([D, B], BF16, tag="sq")
                            nc.vector.tensor_mul(sq[:], big[:, jj, :],
                                                 big[:, jj, :])
                            pending.append((j, sq))
                            if len(pending) > 2:
                                emit_cn2(*pending.popleft())
            while pending:
                emit_cn2(*pending.popleft())

            # inv_cn = 1 / max(sqrt(cn2), 1e-12)
            nc.scalar.activation(cn_tmp[:], cn2[:], AF.Sqrt)
        nc.vector.tensor_scalar_max(cn_tmp[:], cn_tmp[:], 1e-12)
        nc.vector.reciprocal(inv_cn[:], cn_tmp[:])

        # ---- Phase 2: per-sample RMS norm + MLP ----
        with tc.tile_pool(name="pix", bufs=6) as pixp, \
             tc.tile_pool(name="sqp", bufs=3) as sqpp, \
             tc.tile_pool(name="rms", bufs=2) as rmsp, \
             tc.tile_pool(name="invs", bufs=2) as invsp, \
             tc.tile_pool(name="xn", bufs=3) as xnp, \
             tc.tile_pool(name="sh", bufs=2) as shp, \
             tc.tile_pool(name="ot", bufs=3) as otp, \
             tc.tile_pool(name="msps", bufs=2, space="PSUM") as msps, \
             tc.tile_pool(name="sbcps", bufs=2, space="PSUM") as sbcps, \
             tc.tile_pool(name="hps", bufs=2, space="PSUM") as hpsp, \
             tc.tile_pool(name="ops", bufs=2, space="PSUM") as opsp:

            stA = {}
            stB = {}

            def stage_a(s):
                pix = pixp.tile([D, NPIX], BF16, tag="pix")
                nc.sync.dma_start(pix[:], pixT_d.ap()[s])
                sqp = sqpp.tile([D, NPIX], BF16, tag="sqp")
                nc.vector.tensor_mul(sqp[:], pix[:], pix[:])
                ms = msps.tile([1, NPIX], F32, tag="ms")
                nc.tensor.matmul(ms[:], onescol[:], sqp[:])
                stA[s] = (pix, ms)

            def stage_b(s):
                pix, ms = stA.pop(s)
                rms = rmsp.tile([1, NPIX], F32, tag="rms")
                nc.scalar.activation(rms[:], ms[:], AF.Sqrt,
                                     bias=epsb[:], scale=1.0 / D)
                invs = invsp.tile([1, NPIX], BF16, tag="invs")
                with nc.allow_low_precision("bf16 inv-rms broadcast"):
                    nc.vector.reciprocal(invs[:], rms[:])
                sbc = sbcps.tile([D, NPIX], F32, tag="sbc")
                nc.tensor.matmul(sbc[:], nw[:], invs[:])
                xn = xnp.tile([D, NPIX], BF16, tag="xn")
                nc.vector.tensor_mul(xn[:], pix[:], sbc[:])
                stB[s] = (pix, xn)

            def stage_c(s):
                pix, xn = stB.pop(s)
                h = hpsp.tile([D, NPIX], F32, tag="h")
                nc.tensor.matmul(h[:], big[:, 0:D, s], xn[:])
                sh = shp.tile([D, NPIX], BF16, tag="sh")
                if use_silu:
                    nc.scalar.activation(sh[:], h[:], AF.Silu,
                                         scale=inv_cn[:, s:s + 1])
                else:
                    # CoreSim lacks Silu: silu(h*c) = (h*c) * sigmoid(h*c)
                    sg = shp.tile([D, NPIX], BF16, tag="sg")
                    nc.scalar.activation(sg[:], h[:], AF.Sigmoid,
                                         scale=inv_cn[:, s:s + 1])
                    nc.vector.scalar_tensor_tensor(
                        sh[:], h[:], inv_cn[:, s:s + 1], sg[:],
                        op0=mybir.AluOpType.mult, op1=mybir.AluOpType.mult)
                o = opsp.tile([D, NPIX], F32, tag="o")
                nc.tensor.matmul(o[:], big[:, D:2 * D, s], sh[:])
                ot = otp.tile([D, NPIX], F32, tag="ot")
                nc.vector.tensor_add(ot[:], o[:], pix[:])
                nc.sync.dma_start(outT_d.ap()[s], ot[:])

            for s in range(B + 2):
                if s < B:
                    stage_a(s)
                if 1 <= s < B + 1:
                    stage_b(s - 1)
                if s >= 2:
                    stage_c(s - 2)

    nc.compile()
    return nc


def host_prep(pixels, patches, W_pg, b_pg, norm_w):
    bf = ml_dtypes.bfloat16
    pixT = np.ascontiguousarray(
        pixels.astype(bf).transpose(0, 2, 1))              # (BS, D, NPIX)
    patT = np.ascontiguousarray(patches.T.astype(bf))      # (PD, BS)
    # j-major permutation: W[half, k, j*128+i] = W_pg[half*d*d + i*128 + j, k]
    Wp = W_pg.reshape(2, D, D, PD).transpose(0, 3, 2, 1)   # (2, PD, j, i)
    Wp = np.ascontiguousarray(Wp).reshape(2, PD, D * D).astype(bf)
    Bias = np.ascontiguousarray(b_pg.reshape(2, D, D)).astype(np.float32)
    nw = np.ascontiguousarray(norm_w.reshape(1, D)).astype(bf)
    return pixT, patT, Wp, Bias, nw


_NC_CACHE = {}


def _run(pixels, patches, W_pg, b_pg, norm_w, **spmd_kwargs):
    from concourse.bass_utils import run_bass_kernel_spmd

    pixT, patT, Wp, Bias, nw = host_prep(pixels, patches, W_pg, b_pg, norm_w)
    B = pixels.shape[0] // N_CORES
    if B not in _NC_CACHE:
        _NC_CACHE[B] = build_program(B)
    nc = _NC_CACHE[B]

    in_maps = []
    for c in range(N_CORES):
        in_maps.append({
            "pixT": pixT[c * B:(c + 1) * B],
            "patchesT": np.ascontiguousarray(patT[:, c * B:(c + 1) * B]),
            "W": Wp,
            "Bias": Bias,
            "normw": nw,
        })
    res = run_bass_kernel_spmd(nc, in_maps, list(range(N_CORES)), **spmd_kwargs)
    outT = np.concatenate([res.results[c]["outT"] for c in range(N_CORES)], 0)
    return np.ascontiguousarray(outT.transpose(0, 2, 1)), res


def kernel(pixels, patches, W_pg, b_pg, norm_w):
    out, _ = _run(pixels, patches, W_pg, b_pg, norm_w)
    return out


if __name__ == "__main__":
    rng = np.random.default_rng(0)
    inputs = {
        "pixels": rng.standard_normal((BS, NPIX, D), dtype=np.float32),
        "patches": rng.standard_normal((BS, PD), dtype=np.float32),
        "W_pg": (rng.standard_normal((2 * D * D, PD)) * 0.02).astype(np.float32),
        "b_pg": np.zeros((2 * D * D,), np.float32),
        "norm_w": np.ones((D,), np.float32),
    }
    out = kernel(**inputs)
    print(out.shape, out.dtype)


# revision 16
# speedup vs baseline: 2.0319x; 2.0319x over previous
"""NerfHead Trainium2 kernel: per-sample generated 2-layer MLP over pixels.

Sharding: pure data parallel over the batch dim across 8 cores.
Device works in d-major ("transposed") pixel layout; host does all layout
permutations (legal: host only re-stages inputs/outputs, all FLOPs on device).

Structure (per core, B=256 samples):
  Phase 1 (merged): params = W @ patches as 256 j-tiles [128, B] PSUM,
    evacuated (+bias) into big SBUF [128, 256j, B] bf16 (ACT/DVE alternating);
    col-norm^2 of layer1 via masked-ones matmul accumulation -> cn2 [128e, B];
    interleaved: per-sample pixel RMS stats sum(pix^2) stacked by rows into
    ms[2] [128slot, 256n] PSUM via the same masked-ones trick.
  Batch rsqrt: 3 ACT Sqrt + 3 DVE reciprocals total (table switch only here).
  Phase 2 (per sample): identity-column broadcast matmul replicates the
    sample's inv-rms row -> s_bc; xn = pix * norm_w * s_bc (one DVE op);
    mlp1 matmul -> ACT Silu(scale=inv_cn col) fusing the L2 normalize ->
    mlp2 matmul -> residual add -> store (transposed; host untransposes).
"""
import sys
from contextlib import ExitStack

import ml_dtypes
import numpy as np

sys.path.insert(0, "/opt/trn_rl_repo")

import concourse.bass as bass  # noqa: E402
import concourse.tile as tile  # noqa: E402
from concourse import bacc, mybir  # noqa: E402

BF16 = mybir.dt.bfloat16
F32 = mybir.dt.float32
AF = mybir.ActivationFunctionType
MULT = mybir.AluOpType.mult

RMS_EPS = 1.1920928955078125e-07

N_CORES = 8
BS = 2048
NPIX = 256
D = 128
PD = 768  # patch_dim
KC = PD // 128  # 6 contraction chunks
NJ = 2 * D  # 256 j-tiles total (128 per half)
JG = 16  # j-tiles per W slab DMA


def build_program(B, use_silu=True):
    """Build the per-core Bass program for a shard of B samples."""
    assert B % 2 == 0 and B <= 256
    nc = bacc.Bacc("TRN2", target_bir_lowering=False, debug=False,
                   num_devices=N_CORES)

    pixT_d = nc.dram_tensor("pixT", (B, D, NPIX), BF16, kind="ExternalInput")
    patT_d = nc.dram_tensor("patchesT", (PD, B), BF16, kind="ExternalInput")
    w_d = nc.dram_tensor("W", (2, PD, D * D), BF16, kind="ExternalInput")
    bias_d = nc.dram_tensor("Bias", (2, D, D), F32, kind="ExternalInput")
    nwc_d = nc.dram_tensor("normwc", (D, 1), F32, kind="ExternalInput")
    id_d = nc.dram_tensor("ident", (D, D), BF16, kind="ExternalInput")
    outT_d = nc.dram_tensor("outT", (B, D, NPIX), F32, kind="ExternalOutput")

    n_grp = (B + 127) // 128  # ms row-stack groups

    with tile.TileContext(nc) as tc, ExitStack() as ctx:
        const = ctx.enter_context(tc.tile_pool(name="const", bufs=1))
        bigp = ctx.enter_context(tc.tile_pool(name="big", bufs=1))

        # constants / persistent tiles
        pats = []
        for k in range(KC):
            t = const.tile([128, B], BF16, tag=f"pat{k}")
            nc.sync.dma_start(t[:], patT_d.ap()[k * 128:(k + 1) * 128, :])
            pats.append(t)
        bt = const.tile([D, 2, D], F32, tag="bias")
        nc.sync.dma_start(bt[:], bias_d.ap().rearrange("h i j -> i h j"))
        nwc = const.tile([D, 1], F32, tag="normwc")
        nc.sync.dma_start(nwc[:], nwc_d.ap())
        ident = const.tile([D, D], BF16, tag="ident")
        nc.sync.dma_start(ident[:], id_d.ap())
        maskones = const.tile([128, 2 * D + 1], BF16, tag="maskones")
        nc.vector.memset(maskones[:], 0.0)
        nc.vector.memset(maskones[:, D:D + 1], 1.0)
        epsb = const.tile([128, 1], F32, tag="epsb")
        nc.vector.memset(epsb[:], RMS_EPS)
        inv_cn = const.tile([D, B], F32, tag="invcn")
        cn_tmp = const.tile([D, B], F32, tag="cntmp")
        invs = [const.tile([128, NPIX], BF16, tag=f"invs{g}", name=f"invs{g}")
                for g in range(n_grp)]
        rms_t = [const.tile([128, NPIX], F32, tag=f"rms{g}", name=f"rms{g}")
                 for g in range(n_grp)]

        big = bigp.tile([128, NJ, B], BF16, tag="big")

        # ---- Phase 1 (merged): params gen + cn2 + pixel RMS stats ----
        with tc.tile_pool(name="wslab", bufs=2 * KC) as wpool, \
             tc.tile_pool(name="sq1", bufs=3) as sqpool, \
             tc.tile_pool(name="pixa", bufs=3) as pixap, \
             tc.tile_pool(name="sqp", bufs=4) as sqpp, \
             tc.tile_pool(name="mm1ps", bufs=4, space="PSUM") as mm1ps, \
             tc.tile_pool(name="cn2ps", bufs=1, space="PSUM") as cn2ps, \
             tc.tile_pool(name="msps", bufs=1, space="PSUM") as msps:
            cn2 = cn2ps.tile([D, B], F32, tag="cn2")
            ms = [msps.tile([128, NPIX], F32, tag=f"ms{g}", name=f"ms{g}")
                  for g in range(n_grp)]
            pending_cn2 = []
            pending_ms = []
            pixa_cur = [None]

            def emit_cn2(j, sq):
                nc.tensor.matmul(cn2[:], maskones[:, D - j:2 * D - j], sq[:],
                                 start=(j == 0), stop=(j == D - 1))

            def emit_ms(s, sqp):
                g, slot = divmod(s, 128)
                hi = 128 if B >= (g + 1) * 128 else B - g * 128
                nc.tensor.matmul(ms[g][:],
                                 maskones[:, D - slot:2 * D - slot], sqp[:],
                                 start=(slot == 0), stop=(slot == hi - 1))

            def stats_stage(s):
                # pixel pair load (gpsimd-triggered) + square + delayed ms-mm
                if s % 2 == 0:
                    pp = pixap.tile([128, 2, NPIX], BF16, tag="pixa")
                    nc.gpsimd.dma_start(
                        pp[:],
                        pixT_d.ap()[s:s + 2].rearrange("b p n -> p b n"))
                    pixa_cur[0] = pp
                sqp = sqpp.tile([128, NPIX], BF16, tag="sqp")
                nc.vector.tensor_mul(sqp[:], pixa_cur[0][:, s % 2, :],
                                     pixa_cur[0][:, s % 2, :])
                pending_ms.append((s, sqp))
                if len(pending_ms) > 2:
                    emit_ms(*pending_ms.pop(0))

            for half in range(2):
                for jg in range(D // JG):
                    slabs = []
                    for k in range(KC):
                        sl = wpool.tile([128, JG * 128], BF16, tag="wslab")
                        nc.sync.dma_start(
                            sl[:], w_d.ap()[half, k * 128:(k + 1) * 128,
                                            jg * JG * 128:(jg + 1) * JG * 128])
                        slabs.append(sl)
                    for jl in range(JG):
                        j = jg * JG + jl
                        jj = half * D + j
                        ps = mm1ps.tile([D, B], F32, tag="mm1")
                        for k in range(KC):
                            nc.tensor.matmul(
                                ps[:], slabs[k][:, jl * 128:(jl + 1) * 128],
                                pats[k][:], start=(k == 0), stop=(k == KC - 1))
                        # evac + bias, alternating ACT/DVE
                        if jj % 2 == 0:
                            nc.scalar.activation(big[:, jj, :], ps[:],
                                                 AF.Identity,
                                                 bias=bt[:, half, j:j + 1])
                        else:
                            with nc.allow_low_precision("bf16 staging"):
                                nc.vector.tensor_scalar_add(
                                    big[:, jj, :], ps[:],
                                    bt[:, half, j:j + 1])
                        if half == 0:
                            sq = sqpool.tile([D, B], BF16, tag="sq")
                            nc.vector.tensor_mul(sq[:], big[:, jj, :],
                                                 big[:, jj, :])
                            pending_cn2.append((j, sq))
                            if len(pending_cn2) > 2:
                                emit_cn2(*pending_cn2.pop(0))
                        if jj < B:
                            stats_stage(jj)
            for args in pending_cn2:
                emit_cn2(*args)
            for args in pending_ms:
                emit_ms(*args)
            for s in range(NJ, B):  # leftover samples if B > 256 (unused)
                stats_stage(s)

            # ---- batched rsqrt (single ACT table round-trip) ----
            nc.scalar.activation(cn_tmp[:], cn2[:], AF.Sqrt)
            nc.vector.tensor_scalar_max(cn_tmp[:], cn_tmp[:], 1e-12)
            nc.vector.reciprocal(inv_cn[:], cn_tmp[:])
            for g in range(n_grp):
                nc.scalar.activation(rms_t[g][:], ms[g][:], AF.Sqrt,
                                     bias=epsb[:], scale=1.0 / D)
                with nc.allow_low_precision("bf16 inv-rms"):
                    nc.vector.reciprocal(invs[g][:], rms_t[g][:])

        # ---- Phase 2: per-sample MLP ----
        with tc.tile_pool(name="pix", bufs=4) as pixp, \
             tc.tile_pool(name="sh", bufs=2) as shp, \
             tc.tile_pool(name="xn", bufs=3) as xnp, \
             tc.tile_pool(name="ot", bufs=3) as otp, \
             tc.tile_pool(name="sbcps", bufs=2, space="PSUM") as sbcps, \
             tc.tile_pool(name="hps", bufs=2, space="PSUM") as hpsp, \
             tc.tile_pool(name="ops", bufs=2, space="PSUM") as opsp:

            stA = {}
            pixb_cur = [None]
            ot_cur = [None]

            def stage_a(s):
                if s % 2 == 0:
                    pp = pixp.tile([128, 2, NPIX], BF16, tag="pix")
                    nc.gpsimd.dma_start(
                        pp[:],
                        pixT_d.ap()[s:s + 2].rearrange("b p n -> p b n"))
                    pixb_cur[0] = pp
                pix = pixb_cur[0][:, s % 2, :]
                g, slot = divmod(s, 128)
                sbc = sbcps.tile([D, NPIX], F32, tag="sbc")
                nc.tensor.matmul(
                    sbc[:],
                    ident[:, slot:slot + 1].to_broadcast((D, D)),
                    invs[g][:])
                xn = xnp.tile([D, NPIX], BF16, tag="xn")
                nc.vector.scalar_tensor_tensor(xn[:], pix, nwc[:], sbc[:],
                                               op0=MULT, op1=MULT)
                stA[s] = (pix, xn)

            def stage_b(s):
                pix, xn = stA.pop(s)
                h = hpsp.tile([D, NPIX], F32, tag="h")
                nc.tensor.matmul(h[:], big[:, 0:D, s], xn[:])
                sh = shp.tile([D, NPIX], BF16, tag="sh")
                if use_silu:
                    nc.scalar.activation(sh[:], h[:], AF.Silu,
                                         scale=inv_cn[:, s:s + 1])
                else:
                    sg = shp.tile([D, NPIX], BF16, tag="sg")
                    nc.scalar.activation(sg[:], h[:], AF.Sigmoid,
                                         scale=inv_cn[:, s:s + 1])
                    nc.vector.scalar_tensor_tensor(
                        sh[:], h[:], inv_cn[:, s:s + 1], sg[:],
                        op0=MULT, op1=MULT)
                o = opsp.tile([D, NPIX], F32, tag="o")
                nc.tensor.matmul(o[:], big[:, D:2 * D, s], sh[:])
                if s % 2 == 0:
                    ot_cur[0] = otp.tile([128, 2, NPIX], F32, tag="ot",
                                         name="ot")
                ot = ot_cur[0]
                nc.vector.tensor_add(ot[:, s % 2, :], o[:], pix)
                if s % 2 == 1:
                    nc.sync.dma_start(
                        outT_d.ap()[s - 1:s + 1].rearrange("b p n -> p b n"),
                        ot[:])

            for s in range(B + 2):
                if s < B:
                    stage_a(s)
                if s >= 2:
                    stage_b(s - 2)

    nc.compile()
    return nc


def host_prep(pixels, patches, W_pg, b_pg, norm_w):
    bf = ml_dtypes.bfloat16
    pixT = np.ascontiguousarray(
        pixels.astype(bf).transpose(0, 2, 1))              # (BS, D, NPIX)
    patT = np.ascontiguousarray(patches.T.astype(bf))      # (PD, BS)
    # j-major permutation: W[half, k, j*128+i] = W_pg[half*d*d + i*128 + j, k]
    Wp = W_pg.reshape(2, D, D, PD).transpose(0, 3, 2, 1)   # (2, PD, j, i)
    Wp = np.ascontiguousarray(Wp).reshape(2, PD, D * D).astype(bf)
    Bias = np.ascontiguousarray(b_pg.reshape(2, D, D)).astype(np.float32)
    nwc = np.ascontiguousarray(norm_w.reshape(D, 1)).astype(np.float32)
    ident = np.eye(D, dtype=bf)
    return pixT, patT, Wp, Bias, nwc, ident


_NC_CACHE = {}


def _run(pixels, patches, W_pg, b_pg, norm_w, **spmd_kwargs):
    from concourse.bass_utils import run_bass_kernel_spmd

    pixT, patT, Wp, Bias, nwc, ident = host_prep(
        pixels, patches, W_pg, b_pg, norm_w)
    B = pixels.shape[0] // N_CORES
    if B not in _NC_CACHE:
        _NC_CACHE[B] = build_program(B)
    nc = _NC_CACHE[B]

    in_maps = []
    for c in range(N_CORES):
        in_maps.append({
            "pixT": pixT[c * B:(c + 1) * B],
            "patchesT": np.ascontiguousarray(patT[:, c * B:(c + 1) * B]),
            "W": Wp,
            "Bias": Bias,
            "normwc": nwc,
            "ident": ident,
        })
    res = run_bass_kernel_spmd(nc, in_maps, list(range(N_CORES)), **spmd_kwargs)
    outT = np.concatenate([res.results[c]["outT"] for c in range(N_CORES)], 0)
    return np.ascontiguousarray(outT.transpose(0, 2, 1)), res


def kernel(pixels, patches, W_pg, b_pg, norm_w):
    out, _ = _run(pixels, patches, W_pg, b_pg, norm_w)
    return out


if __name__ == "__main__":
    rng = np.random.default_rng(0)
    inputs = {
        "pixels": rng.standard_normal((BS, NPIX, D), dtype=np.float32),
        "patches": rng.standard_normal((BS, PD), dtype=np.float32),
        "W_pg": (rng.standard_normal((2 * D * D, PD)) * 0.02).astype(np.float32),
        "b_pg": np.zeros((2 * D * D,), np.float32),
        "norm_w": np.ones((D,), np.float32),
    }
    out = kernel(**inputs)
    print(out.shape, out.dtype)
